# revision 27
# baseline (speedup 1.0000x reference)
"""Trainium2 Bass kernel for single-head causal attention.

Problem: x:[4,2048,768], Wq/Wk/Wv:[768,768] (torch-Linear layout, y = x @ W.T),
out = causal_softmax(q k^T / sqrt(768)) @ v, all float32.

Sharding (8 NeuronCores, no collectives):
  - core pair (2b, 2b+1) handles batch b.
  - per batch, the 16 query tiles of 128 rows are split between the pair as
    {0,3,4,7,8,11,12,15} and {1,2,5,6,9,10,13,14}. Sorted by causal length
    those are {1,4,5,8,9,12,13,16} and {2,3,6,7,10,11,14,15} key-tiles, so
    both sides fit the same static per-slot key budget {2,4,...,16}: the one
    SPMD graph processes 8 query tiles whose key ranges are padded by at most
    one 128-tile (+6% flops) and the pad/diagonal is handled by a host-
    provided additive mask.
  - scores are computed TRANSPOSED (kT stationary, qT moving -> [k, q] in
    PSUM): exp output probsT[k, q] is directly the stationary operand the
    probs @ x matmul needs, so no probability transposes at all.  The softmax
    denominator comes from a ones-column appended to the row-major x operand
    (Y[:, 768] = sum of probs), and 1/l is folded into the Y -> SBUF copies.
  - out = (probs @ x) @ Wv^T (saves the full-seq V projection); only the
    [q, d] -> [d, q] transpose of Y runs on the TensorEngine (6 per slot).
  - host pre-transposes/packs inputs to bf16 and supplies the 128x128
    identity, so the device never builds constants on the critical path; PE
    warm-up runs on a memset-zeros tile starting immediately.
"""

import math
import os
import sys

import numpy as np

if not any(os.path.isdir(os.path.join(p, "concourse")) for p in sys.path):
    sys.path.insert(0, "/opt/trn_rl_repo")

import concourse.bass as bass  # noqa: E402
import concourse.mybir as mybir  # noqa: E402
from concourse import bacc, tile  # noqa: E402
from concourse.bass_utils import run_bass_kernel_spmd  # noqa: E402

import ml_dtypes  # noqa: E402

B, S, D = 4, 2048, 768
P = 128
NT = S // P          # 16 key tiles per batch
DC = D // P          # 6 contraction chunks
NSLOT = 8            # query tiles per core
QROWS = NSLOT * P    # 1024 query rows per core
N_CORES = 8
SCALE = 1.0 / math.sqrt(D)
XW = D + 1           # row-major x width incl. the ones column

SIDE_A = [0, 3, 4, 7, 8, 11, 12, 15]   # causal lengths 1,4,5,8,9,12,13,16
SIDE_B = [1, 2, 5, 6, 9, 10, 13, 14]   # causal lengths 2,3,6,7,10,11,14,15
CAP = [2, 4, 6, 8, 10, 12, 14, 16]     # static key tiles per slot (>= real)

BF16 = ml_dtypes.bfloat16

_NC = None


def ktw(kt):
    """Query-column span of key tile kt: slots s >= kt//2 need it."""
    return QROWS - (kt // 2) * P


def build():
    """Build + compile the single SPMD graph run by all 8 cores."""
    f32 = mybir.dt.float32
    bf16 = mybir.dt.bfloat16

    nc = bacc.Bacc("TRN2", target_bir_lowering=False, debug=False,
                   num_devices=N_CORES)

    # inputs come pre-packed as [P, chunk, width] (host layout transform)
    xq_d = nc.dram_tensor("xqT", [P, 2, DC, 512], bf16,
                          kind="ExternalInput").ap()
    xkv_d = nc.dram_tensor("xkvT", [P, DC, S], bf16,
                           kind="ExternalInput").ap()
    xv_d = nc.dram_tensor("xvR", [P, NT, XW], bf16,
                          kind="ExternalInput").ap()
    wq_d = nc.dram_tensor("wqT", [P, 3, DC, 256], bf16,
                          kind="ExternalInput").ap()
    wk_d = nc.dram_tensor("wkT", [P, DC, D], bf16, kind="ExternalInput").ap()
    wv_d = nc.dram_tensor("wvT", [P, DC, D], bf16, kind="ExternalInput").ap()
    mask_d = nc.dram_tensor("maskT", [P, NSLOT, 2, P], bf16,
                            kind="ExternalInput").ap()
    id_d = nc.dram_tensor("ident", [P, P], bf16, kind="ExternalInput").ap()
    out_d = nc.dram_tensor("out", [QROWS, D], bf16,
                           kind="ExternalOutput").ap()

    with tile.TileContext(nc) as tc:
        with (
            tc.tile_pool(name="const", bufs=1) as const,
            tc.tile_pool(name="osb", bufs=2) as osb_pool,
            tc.tile_pool(name="yt", bufs=2) as yt_pool,
            tc.tile_pool(name="small", bufs=2) as small,
            tc.tile_pool(name="ps_s", bufs=2, space="PSUM") as ps_s,
            tc.tile_pool(name="ps_tr", bufs=2, space="PSUM") as ps_tr,
            tc.tile_pool(name="ps_o", bufs=2, space="PSUM") as ps_o,
        ):
            # ---- persistent SBUF tensors, halves so DMA overlaps compute
            HC = DC // 2
            wq_p = [[const.tile([P, HC, 256], bf16, tag=f"wqp{i}_{h}",
                                name=f"wqp{i}_{h}") for h in range(2)]
                    for i in range(3)]
            wk_h = [const.tile([P, HC, D], bf16, tag=f"wkh{h}", name=f"wkh{h}")
                    for h in range(2)]
            wv_h = [const.tile([P, HC, D], bf16, tag=f"wvh{h}", name=f"wvh{h}")
                    for h in range(2)]
            xq_g = [[const.tile([P, HC, 512], bf16, tag=f"xqg{g}_{h}",
                                name=f"xqg{g}_{h}") for h in range(2)]
                    for g in range(2)]
            xkv_h = [const.tile([P, HC, S], bf16, tag=f"xkvh{h}",
                                name=f"xkvh{h}") for h in range(2)]

            def chunk(tiles, dc):
                return tiles[dc // HC][:, dc % HC, :]

            wk_c = [chunk(wk_h, c) for c in range(DC)]
            wv_c = [chunk(wv_h, c) for c in range(DC)]
            xkv_c = [chunk(xkv_h, c) for c in range(DC)]
            mask_sb = const.tile([P, NSLOT, 2, P], bf16, tag="mask")
            ident = const.tile([P, P], bf16, tag="ident")
            zeros = const.tile([P, 512], bf16, tag="zeros")
            qt_sb = const.tile([P, DC, QROWS], bf16, tag="qt")
            kt_g = [const.tile([P, DC, 512], bf16, tag=f"ktg{g}",
                               name=f"ktg{g}") for g in range(S // 512)]
            xv_h = [const.tile([P, NT // 2, XW], bf16, tag=f"xvh{h}",
                               name=f"xvh{h}") for h in range(2)]
            probsT = [const.tile([P, ktw(kt)], bf16, tag=f"pT{kt}",
                                 name=f"pT{kt}") for kt in range(NT)]

            # priority-ordered input DMAs, d-chunk halves so the first
            # Q-projection matmuls start on ~0.5MB of input
            nc.sync.dma_start(out=xq_g[0][0][:, :, :],
                              in_=xq_d[:, 0, 0:HC, :])
            nc.sync.dma_start(out=wq_p[0][0][:, :, :],
                              in_=wq_d[:, 0, 0:HC, :])
            nc.sync.dma_start(out=xq_g[0][1][:, :, :],
                              in_=xq_d[:, 0, HC:DC, :])
            nc.sync.dma_start(out=wq_p[0][1][:, :, :],
                              in_=wq_d[:, 0, HC:DC, :])
            for i in range(1, 3):
                for h in range(2):
                    nc.sync.dma_start(out=wq_p[i][h][:, :, :],
                                      in_=wq_d[:, i, h * HC:(h + 1) * HC, :])
            for h in range(2):
                nc.sync.dma_start(out=xq_g[1][h][:, :, :],
                                  in_=xq_d[:, 1, h * HC:(h + 1) * HC, :])
            nc.sync.dma_start(out=ident[:, :], in_=id_d[:, :])

            # HAM warm-up on a zeroed tile: PE busy from ~0.3us so the real
            # matmuls run at 2.4GHz as soon as their inputs land.
            # 38 x 128-wide on zeros: ~3.4us cold flips HAM to 2.4GHz, the
            # rest bridges to the first input DMA landing
            nc.gpsimd.memset(zeros[:, :], 0.0)
            warm = ps_s.tile([P, 512], f32, tag="mm512", name="warm")
            for _ in range(38):
                nc.tensor.matmul(warm[:, 0:P], zeros[:, 0:P], zeros[:, 0:P],
                                 start=True, stop=True)

            # ---- qT[o,q] projection (group-major: starts on first DMAs)
            for g in range(QROWS // 512):
                for oc in range(DC):
                    ps = ps_s.tile([P, 512], f32, tag="mm512")
                    for dc in range(DC):
                        nc.tensor.matmul(
                            ps[:, :],
                            wq_p[oc // 2][dc // HC][:, dc % HC,
                                                    (oc % 2) * P:
                                                    (oc % 2 + 1) * P],
                            xq_g[g][dc // HC][:, dc % HC, :],
                            start=(dc == 0), stop=(dc == DC - 1))
                    nc.scalar.copy(qt_sb[:, oc, g * 512:(g + 1) * 512],
                                   ps[:, :])

            for h in range(2):
                nc.sync.dma_start(out=xkv_h[h][:, :, :],
                                  in_=xkv_d[:, h * HC:(h + 1) * HC, :])
                nc.sync.dma_start(out=wk_h[h][:, :, :],
                                  in_=wk_d[:, h * HC:(h + 1) * HC, :])

            def gsplit(w):
                """Split w (multiple of 128) into <=512 parts, balanced in
                128-multiples: narrow matmuls are LDWEIGHTS-bound, so 384+384
                beats 512+256."""
                parts = (w + 511) // 512
                tiles = w // P
                out, acc = [], 0
                for i in range(parts):
                    t = (tiles * (i + 1)) // parts - (tiles * i) // parts
                    out.append((acc, t * P))
                    acc += t * P
                return out

            def emit_scoresT(kt):
                """scoresT[k, q] for key tile kt over q-cols [qlo, QROWS)."""
                qlo = (kt // 2) * P
                w = QROWS - qlo
                sm, j = kt // 2, kt % 2   # the one masked slot for this kt
                for off0, cw in gsplit(w):
                    c0 = qlo + off0
                    ps = ps_s.tile([P, 512], f32, tag="mm512",
                                   name=f"st{kt}_{off0}")
                    for oc in range(DC):
                        nc.tensor.matmul(
                            ps[:, :cw],
                            kt_g[kt // 4][:, oc, (kt % 4) * P:(kt % 4 + 1) * P],
                            qt_sb[:, oc, c0:c0 + cw],
                            start=(oc == 0), stop=(oc == DC - 1))
                    if c0 <= sm * P < c0 + cw:
                        off = sm * P - c0
                        nc.vector.tensor_add(ps[:, off:off + P],
                                             ps[:, off:off + P],
                                             mask_sb[:, sm, j, :])
                    nc.scalar.activation(
                        probsT[kt][:, c0 - qlo:c0 - qlo + cw], ps[:, :cw],
                        mybir.ActivationFunctionType.Exp, scale=SCALE)

            def emit_rest(s):
                """AV + (Y @ Wv^T)/l + output DMA for slot s."""
                L = CAP[s]
                # PSUM bank0 = Y[384:768] + l (385 cols), bank1 = Y[0:384]:
                # balanced 385/384 groups, neither crossing a bank boundary
                out_ps = ps_o.tile([P, 896], f32, tag="mmout",
                                   name=f"ops{s}")
                for kt in range(L):
                    pT = probsT[kt][:, s * P - (kt // 2) * P:
                                    (s + 1) * P - (kt // 2) * P]
                    xv = xv_h[kt // (NT // 2)][:, kt % (NT // 2), :]
                    nc.tensor.matmul(out_ps[:, 0:385],
                                     pT, xv[:, 384:XW],
                                     start=(kt == 0), stop=(kt == L - 1))
                    nc.tensor.matmul(out_ps[:, 512:896],
                                     pT, xv[:, 0:384],
                                     start=(kt == 0), stop=(kt == L - 1))
                rinv = small.tile([P, 1], f32, tag="rinv", name=f"rinv{s}")
                nc.vector.reciprocal(rinv[:, :], out_ps[:, 384:385])
                # unnormalized Y in bf16 (plain copies keep the chain short);
                # 1/l is applied on the final output copies instead
                y_hi = osb_pool.tile([P, 384], bf16, tag="yhi", name=f"yhi{s}")
                y_lo = osb_pool.tile([P, 384], bf16, tag="ylo", name=f"ylo{s}")
                nc.vector.tensor_copy(y_hi[:, :], out_ps[:, 0:384])
                nc.scalar.copy(y_lo[:, 0:192], out_ps[:, 512:704])
                nc.vector.tensor_copy(y_lo[:, 192:384], out_ps[:, 704:896])
                ytT = yt_pool.tile([P, D], bf16, tag="ytT", name=f"ytT{s}")
                for kg in range(2):      # hi half first: its operand lands
                    tp = ps_tr.tile([P, 384], bf16, tag="tr",
                                    name=f"ytp{s}")  # first
                    ysrc_t = y_hi if kg == 0 else y_lo
                    base = 3 if kg == 0 else 0
                    for j in range(3):
                        nc.tensor.transpose(tp[:, j * P:(j + 1) * P],
                                            ysrc_t[:, j * P:(j + 1) * P],
                                            ident[:, :])
                    nc.vector.tensor_copy(ytT[:, base * P:base * P + 384],
                                          tp[:, 0:384])
                out2_ps = ps_o.tile([P, 896], f32, tag="mmout",
                                    name=f"o2ps{s}")
                out_sb = osb_pool.tile([P, D], bf16, tag="osb", name=f"osb{s}")
                # accumulate hi chunks (3,4,5) first: they transpose first,
                # so the final matmuls start before the lo half is copied
                dcs = [3, 4, 5, 0, 1, 2]
                for i, dc in enumerate(dcs):
                    nc.tensor.matmul(out2_ps[:, 0:384],
                                     ytT[:, dc * P:(dc + 1) * P],
                                     wv_c[dc][:, 384:D],
                                     start=(i == 0), stop=(i == DC - 1))
                nc.scalar.mul(out_sb[:, 384:D], out2_ps[:, 0:384],
                              rinv[:, :])
                nc.sync.dma_start(out=out_d[s * P:(s + 1) * P, 384:D],
                                  in_=out_sb[:, 384:D])
                for i, dc in enumerate(dcs):
                    nc.tensor.matmul(out2_ps[:, 512:896],
                                     ytT[:, dc * P:(dc + 1) * P],
                                     wv_c[dc][:, 0:384],
                                     start=(i == 0), stop=(i == DC - 1))
                nc.scalar.mul(out_sb[:, 0:192], out2_ps[:, 512:704],
                              rinv[:, :])
                nc.vector.tensor_scalar_mul(out_sb[:, 192:384],
                                            out2_ps[:, 704:896], rinv[:, :])
                nc.sync.dma_start(out=out_d[s * P:(s + 1) * P, 0:384],
                                  in_=out_sb[:, 0:384])

            # kT[o,k] projection group-major, scoresT + attention interleaved
            for g in range(S // 512):
                for oc in range(DC):
                    ps = ps_s.tile([P, 512], f32, tag="mm512",
                                   name=f"ktps{g}_{oc}")
                    for dc in range(DC):
                        nc.tensor.matmul(
                            ps[:, :],
                            wk_c[dc][:, oc * P:(oc + 1) * P],
                            xkv_c[dc][:, g * 512:(g + 1) * 512],
                            start=(dc == 0), stop=(dc == DC - 1))
                    nc.scalar.copy(kt_g[g][:, oc, :], ps[:, :])
                if g == 0:
                    nc.sync.dma_start(out=mask_sb[:, :, :, :],
                                      in_=mask_d[:, :, :, :])
                    for h in range(2):
                        nc.sync.dma_start(out=wv_h[h][:, :, :],
                                          in_=wv_d[:, h * HC:(h + 1) * HC, :])
                        nc.sync.dma_start(
                            out=xv_h[h][:, :, :],
                            in_=xv_d[:, h * (NT // 2):(h + 1) * (NT // 2), :])
                for kt in range(4 * g, 4 * g + 4):
                    emit_scoresT(kt)
                    if kt % 2 == 1:
                        emit_rest(kt // 2)

    nc.compile()
    return nc


def _pack(matT):
    """[D, W] (transposed operand) -> [P, DC, W] chunk layout, bf16."""
    d, w = matT.shape
    return np.ascontiguousarray(
        matT.reshape(d // P, P, w).transpose(1, 0, 2)).astype(BF16)


def shard_inputs(x, Wq, Wk, Wv):
    x = np.asarray(x, dtype=np.float32)
    wqT = _pack(np.asarray(Wq, np.float32).T)            # [P, DC, D]
    wqT = np.ascontiguousarray(                          # [P, 3, DC, 256]
        wqT.reshape(P, DC, 3, 256).transpose(0, 2, 1, 3))
    wkT = _pack(np.asarray(Wk, np.float32).T)
    wvT = _pack(np.asarray(Wv, np.float32).T)
    ident = np.eye(P, dtype=BF16)
    in_maps = []
    for c in range(N_CORES):
        b, side = divmod(c, 2)
        qtiles = SIDE_A if side == 0 else SIDE_B
        xb = x[b]                                    # [S, D]
        xkvT = _pack(np.ascontiguousarray(xb.T))
        xvR = np.empty((NT, P, XW), BF16)            # row-major + ones col
        xvR[:, :, :D] = xb.astype(BF16).reshape(NT, P, D)
        xvR[:, :, D] = BF16(1.0)
        xvR = np.ascontiguousarray(xvR.transpose(1, 0, 2))   # [P, NT, XW]
        xq = np.concatenate([xb[t * P:(t + 1) * P] for t in qtiles], axis=0)
        xqT = _pack(np.ascontiguousarray(xq.T))          # [P, DC, QROWS]
        xqT = np.ascontiguousarray(                      # [P, 2, DC, 512]
            xqT.reshape(P, DC, 2, 512).transpose(0, 2, 1, 3))
        # transposed mask [k-part, slot, which-tile, q-col]
        mask = np.empty((NSLOT, 2, P, P), np.float32)
        for s, t in enumerate(qtiles):
            qidx = t * P + np.arange(P)[None, :]         # query global
            for j in range(2):
                kt = CAP[s] - 2 + j
                kidx = kt * P + np.arange(P)[:, None]    # key global
                mask[s, j] = np.where(kidx <= qidx, 0.0, -1e30)
        # dram layout [P, NSLOT, 2, P]
        mask = np.ascontiguousarray(
            mask.transpose(2, 0, 1, 3)).astype(BF16)
        in_maps.append({"xqT": xqT, "xkvT": xkvT, "xvR": xvR, "wqT": wqT,
                        "wkT": wkT, "wvT": wvT, "maskT": mask,
                        "ident": ident})
    return in_maps


def unshard(results):
    out = np.empty((B, S, D), np.float32)
    for c in range(N_CORES):
        b, side = divmod(c, 2)
        qtiles = SIDE_A if side == 0 else SIDE_B
        oc = np.asarray(results[c]["out"]).astype(np.float32)
        for s, t in enumerate(qtiles):
            out[b, t * P:(t + 1) * P] = oc[s * P:(s + 1) * P]
    return out


def run(inputs, trace=False, trace_cores=None):
    """Run on hardware; returns (output, BassKernelResults)."""
    global _NC
    if _NC is None:
        _NC = build()
    in_maps = shard_inputs(inputs["x"], inputs["Wq"], inputs["Wk"],
                           inputs["Wv"])
    res = run_bass_kernel_spmd(_NC, in_maps, core_ids=list(range(N_CORES)),
                               trace=trace, trace_cores=trace_cores)
    return unshard(res.results), res


def kernel(x, Wq, Wk, Wv):
    out, _ = run({"x": x, "Wq": Wq, "Wk": Wk, "Wv": Wv})
    return out


# revision 28
# speedup vs baseline: 1.0016x; 1.0016x over previous
"""Trainium2 Bass kernel for single-head causal attention.

Problem: x:[4,2048,768], Wq/Wk/Wv:[768,768] (torch-Linear layout, y = x @ W.T),
out = causal_softmax(q k^T / sqrt(768)) @ v, all float32.

Sharding (8 NeuronCores, no collectives):
  - core pair (2b, 2b+1) handles batch b.
  - per batch, the 16 query tiles of 128 rows are split between the pair as
    {0,3,4,7,8,11,12,15} and {1,2,5,6,9,10,13,14}. Sorted by causal length
    those are {1,4,5,8,9,12,13,16} and {2,3,6,7,10,11,14,15} key-tiles, so
    both sides fit the same static per-slot key budget {2,4,...,16}: the one
    SPMD graph processes 8 query tiles whose key ranges are padded by at most
    one 128-tile (+6% flops) and the pad/diagonal is handled by a host-
    provided additive mask.
  - scores are computed TRANSPOSED (kT stationary, qT moving -> [k, q] in
    PSUM): exp output probsT[k, q] is directly the stationary operand the
    probs @ x matmul needs, so no probability transposes at all.  The softmax
    denominator comes from a ones-column appended to the row-major x operand
    (Y[:, 768] = sum of probs), and 1/l is folded into the Y -> SBUF copies.
  - out = (probs @ x) @ Wv^T (saves the full-seq V projection); only the
    [q, d] -> [d, q] transpose of Y runs on the TensorEngine (6 per slot).
  - host pre-transposes/packs inputs to bf16 and supplies the 128x128
    identity, so the device never builds constants on the critical path; PE
    warm-up runs on a memset-zeros tile starting immediately.
"""

import math
import os
import sys

import numpy as np

if not any(os.path.isdir(os.path.join(p, "concourse")) for p in sys.path):
    sys.path.insert(0, "/opt/trn_rl_repo")

import concourse.bass as bass  # noqa: E402
import concourse.mybir as mybir  # noqa: E402
from concourse import bacc, tile  # noqa: E402
from concourse.bass_utils import run_bass_kernel_spmd  # noqa: E402

import ml_dtypes  # noqa: E402

B, S, D = 4, 2048, 768
P = 128
NT = S // P          # 16 key tiles per batch
DC = D // P          # 6 contraction chunks
NSLOT = 8            # query tiles per core
QROWS = NSLOT * P    # 1024 query rows per core
N_CORES = 8
SCALE = 1.0 / math.sqrt(D)
XW = D + 1           # row-major x width incl. the ones column

SIDE_A = [0, 3, 4, 7, 8, 11, 12, 15]   # causal lengths 1,4,5,8,9,12,13,16
SIDE_B = [1, 2, 5, 6, 9, 10, 13, 14]   # causal lengths 2,3,6,7,10,11,14,15
CAP = [2, 4, 6, 8, 10, 12, 14, 16]     # static key tiles per slot (>= real)

BF16 = ml_dtypes.bfloat16

_NC = None


def ktw(kt):
    """Query-column span of key tile kt: slots s >= kt//2 need it."""
    return QROWS - (kt // 2) * P


def build():
    """Build + compile the single SPMD graph run by all 8 cores."""
    f32 = mybir.dt.float32
    bf16 = mybir.dt.bfloat16

    nc = bacc.Bacc("TRN2", target_bir_lowering=False, debug=False,
                   num_devices=N_CORES)

    # inputs come pre-packed as [P, chunk, width] (host layout transform)
    xq_d = nc.dram_tensor("xqT", [P, 2, DC, 512], bf16,
                          kind="ExternalInput").ap()
    xkv_d = nc.dram_tensor("xkvT", [P, DC, S], bf16,
                           kind="ExternalInput").ap()
    xv_d = nc.dram_tensor("xvR", [P, NT, XW], bf16,
                          kind="ExternalInput").ap()
    wq_d = nc.dram_tensor("wqT", [P, 3, DC, 256], bf16,
                          kind="ExternalInput").ap()
    wk_d = nc.dram_tensor("wkT", [P, DC, D], bf16, kind="ExternalInput").ap()
    wv_d = nc.dram_tensor("wvT", [P, DC, D], bf16, kind="ExternalInput").ap()
    mask_d = nc.dram_tensor("maskT", [P, NSLOT, 2, P], bf16,
                            kind="ExternalInput").ap()
    id_d = nc.dram_tensor("ident", [P, P], bf16, kind="ExternalInput").ap()
    out_d = nc.dram_tensor("out", [QROWS, D], bf16,
                           kind="ExternalOutput").ap()

    with tile.TileContext(nc) as tc:
        with (
            tc.tile_pool(name="const", bufs=1) as const,
            tc.tile_pool(name="osb", bufs=2) as osb_pool,
            tc.tile_pool(name="yt", bufs=2) as yt_pool,
            tc.tile_pool(name="small", bufs=2) as small,
            tc.tile_pool(name="ps_s", bufs=2, space="PSUM") as ps_s,
            tc.tile_pool(name="ps_tr", bufs=2, space="PSUM") as ps_tr,
            tc.tile_pool(name="ps_o", bufs=2, space="PSUM") as ps_o,
        ):
            # ---- persistent SBUF tensors, halves so DMA overlaps compute
            HC = DC // 2
            wq_p = [const.tile([P, DC, 256], bf16, tag=f"wqp{i}",
                               name=f"wqp{i}") for i in range(3)]
            wk_h = [const.tile([P, HC, D], bf16, tag=f"wkh{h}", name=f"wkh{h}")
                    for h in range(2)]
            wv_h = [const.tile([P, HC, D], bf16, tag=f"wvh{h}", name=f"wvh{h}")
                    for h in range(2)]
            xq_g = [const.tile([P, DC, 512], bf16, tag=f"xqg{g}",
                               name=f"xqg{g}") for g in range(2)]
            xkv_h = [const.tile([P, HC, S], bf16, tag=f"xkvh{h}",
                                name=f"xkvh{h}") for h in range(2)]

            def chunk(tiles, dc):
                return tiles[dc // HC][:, dc % HC, :]

            wk_c = [chunk(wk_h, c) for c in range(DC)]
            wv_c = [chunk(wv_h, c) for c in range(DC)]
            xkv_c = [chunk(xkv_h, c) for c in range(DC)]
            mask_sb = const.tile([P, NSLOT, 2, P], bf16, tag="mask")
            ident = const.tile([P, P], bf16, tag="ident")
            zeros = const.tile([P, 512], bf16, tag="zeros")
            qt_sb = const.tile([P, DC, QROWS], bf16, tag="qt")
            kt_g = [const.tile([P, DC, 512], bf16, tag=f"ktg{g}",
                               name=f"ktg{g}") for g in range(S // 512)]
            xv_h = [const.tile([P, NT // 2, XW], bf16, tag=f"xvh{h}",
                               name=f"xvh{h}") for h in range(2)]
            probsT = [const.tile([P, ktw(kt)], bf16, tag=f"pT{kt}",
                                 name=f"pT{kt}") for kt in range(NT)]

            # priority-ordered input DMAs: Q-projection group first
            nc.sync.dma_start(out=xq_g[0][:, :, :], in_=xq_d[:, 0, :, :])
            for i in range(3):
                nc.sync.dma_start(out=wq_p[i][:, :, :], in_=wq_d[:, i, :, :])
            nc.sync.dma_start(out=xq_g[1][:, :, :], in_=xq_d[:, 1, :, :])
            nc.sync.dma_start(out=ident[:, :], in_=id_d[:, :])

            # HAM warm-up on a zeroed tile: PE busy from ~0.3us so the real
            # matmuls run at 2.4GHz as soon as their inputs land.
            # 42 x 128-wide on zeros: ~3.4us cold flips HAM to 2.4GHz, the
            # rest bridges to the first input DMA landing (~12.5us)
            nc.gpsimd.memset(zeros[:, :], 0.0)
            warm = ps_s.tile([P, 512], f32, tag="mm512", name="warm")
            for _ in range(42):
                nc.tensor.matmul(warm[:, 0:P], zeros[:, 0:P], zeros[:, 0:P],
                                 start=True, stop=True)

            # ---- qT[o,q] projection (group-major: starts on first DMAs)
            for g in range(QROWS // 512):
                for oc in range(DC):
                    ps = ps_s.tile([P, 512], f32, tag="mm512")
                    for dc in range(DC):
                        nc.tensor.matmul(
                            ps[:, :],
                            wq_p[oc // 2][:, dc,
                                          (oc % 2) * P:(oc % 2 + 1) * P],
                            xq_g[g][:, dc, :],
                            start=(dc == 0), stop=(dc == DC - 1))
                    nc.scalar.copy(qt_sb[:, oc, g * 512:(g + 1) * 512],
                                   ps[:, :])

            for h in range(2):
                nc.sync.dma_start(out=xkv_h[h][:, :, :],
                                  in_=xkv_d[:, h * HC:(h + 1) * HC, :])
                nc.sync.dma_start(out=wk_h[h][:, :, :],
                                  in_=wk_d[:, h * HC:(h + 1) * HC, :])

            def gsplit(w):
                """Split w (multiple of 128) into <=512 parts, balanced in
                128-multiples: narrow matmuls are LDWEIGHTS-bound, so 384+384
                beats 512+256."""
                parts = (w + 511) // 512
                tiles = w // P
                out, acc = [], 0
                for i in range(parts):
                    t = (tiles * (i + 1)) // parts - (tiles * i) // parts
                    out.append((acc, t * P))
                    acc += t * P
                return out

            def emit_scoresT(kt):
                """scoresT[k, q] for key tile kt over q-cols [qlo, QROWS)."""
                qlo = (kt // 2) * P
                w = QROWS - qlo
                sm, j = kt // 2, kt % 2   # the one masked slot for this kt
                for off0, cw in gsplit(w):
                    c0 = qlo + off0
                    ps = ps_s.tile([P, 512], f32, tag="mm512",
                                   name=f"st{kt}_{off0}")
                    for oc in range(DC):
                        nc.tensor.matmul(
                            ps[:, :cw],
                            kt_g[kt // 4][:, oc, (kt % 4) * P:(kt % 4 + 1) * P],
                            qt_sb[:, oc, c0:c0 + cw],
                            start=(oc == 0), stop=(oc == DC - 1))
                    if c0 <= sm * P < c0 + cw:
                        off = sm * P - c0
                        nc.vector.tensor_add(ps[:, off:off + P],
                                             ps[:, off:off + P],
                                             mask_sb[:, sm, j, :])
                    nc.scalar.activation(
                        probsT[kt][:, c0 - qlo:c0 - qlo + cw], ps[:, :cw],
                        mybir.ActivationFunctionType.Exp, scale=SCALE)

            def emit_rest(s):
                """AV + (Y @ Wv^T)/l + output DMA for slot s."""
                L = CAP[s]
                # PSUM bank0 = Y[384:768] + l (385 cols), bank1 = Y[0:384]:
                # balanced 385/384 groups, neither crossing a bank boundary
                out_ps = ps_o.tile([P, 896], f32, tag="mmout",
                                   name=f"ops{s}")
                for kt in range(L):
                    pT = probsT[kt][:, s * P - (kt // 2) * P:
                                    (s + 1) * P - (kt // 2) * P]
                    xv = xv_h[kt // (NT // 2)][:, kt % (NT // 2), :]
                    nc.tensor.matmul(out_ps[:, 0:385],
                                     pT, xv[:, 384:XW],
                                     start=(kt == 0), stop=(kt == L - 1))
                    nc.tensor.matmul(out_ps[:, 512:896],
                                     pT, xv[:, 0:384],
                                     start=(kt == 0), stop=(kt == L - 1))
                rinv = small.tile([P, 1], f32, tag="rinv", name=f"rinv{s}")
                nc.vector.reciprocal(rinv[:, :], out_ps[:, 384:385])
                # unnormalized Y in bf16 (plain copies keep the chain short);
                # 1/l is applied on the final output copies instead
                y_hi = osb_pool.tile([P, 384], bf16, tag="yhi", name=f"yhi{s}")
                y_lo = osb_pool.tile([P, 384], bf16, tag="ylo", name=f"ylo{s}")
                nc.vector.tensor_copy(y_hi[:, :], out_ps[:, 0:384])
                nc.scalar.copy(y_lo[:, 0:192], out_ps[:, 512:704])
                nc.vector.tensor_copy(y_lo[:, 192:384], out_ps[:, 704:896])
                ytT = yt_pool.tile([P, D], bf16, tag="ytT", name=f"ytT{s}")
                for kg in range(2):      # hi half first: its operand lands
                    tp = ps_tr.tile([P, 384], bf16, tag="tr",
                                    name=f"ytp{s}")  # first
                    ysrc_t = y_hi if kg == 0 else y_lo
                    base = 3 if kg == 0 else 0
                    for j in range(3):
                        nc.tensor.transpose(tp[:, j * P:(j + 1) * P],
                                            ysrc_t[:, j * P:(j + 1) * P],
                                            ident[:, :])
                    nc.vector.tensor_copy(ytT[:, base * P:base * P + 384],
                                          tp[:, 0:384])
                out2_ps = ps_o.tile([P, 896], f32, tag="mmout",
                                    name=f"o2ps{s}")
                out_sb = osb_pool.tile([P, D], bf16, tag="osb", name=f"osb{s}")
                # accumulate hi chunks (3,4,5) first: they transpose first,
                # so the final matmuls start before the lo half is copied
                dcs = [3, 4, 5, 0, 1, 2]
                for i, dc in enumerate(dcs):
                    nc.tensor.matmul(out2_ps[:, 0:384],
                                     ytT[:, dc * P:(dc + 1) * P],
                                     wv_c[dc][:, 384:D],
                                     start=(i == 0), stop=(i == DC - 1))
                nc.scalar.mul(out_sb[:, 384:D], out2_ps[:, 0:384],
                              rinv[:, :])
                nc.sync.dma_start(out=out_d[s * P:(s + 1) * P, 384:D],
                                  in_=out_sb[:, 384:D])
                for i, dc in enumerate(dcs):
                    nc.tensor.matmul(out2_ps[:, 512:896],
                                     ytT[:, dc * P:(dc + 1) * P],
                                     wv_c[dc][:, 0:384],
                                     start=(i == 0), stop=(i == DC - 1))
                nc.scalar.mul(out_sb[:, 0:192], out2_ps[:, 512:704],
                              rinv[:, :])
                nc.vector.tensor_scalar_mul(out_sb[:, 192:384],
                                            out2_ps[:, 704:896], rinv[:, :])
                nc.sync.dma_start(out=out_d[s * P:(s + 1) * P, 0:384],
                                  in_=out_sb[:, 0:384])

            # kT[o,k] projection group-major, scoresT + attention interleaved
            for g in range(S // 512):
                for oc in range(DC):
                    ps = ps_s.tile([P, 512], f32, tag="mm512",
                                   name=f"ktps{g}_{oc}")
                    for dc in range(DC):
                        nc.tensor.matmul(
                            ps[:, :],
                            wk_c[dc][:, oc * P:(oc + 1) * P],
                            xkv_c[dc][:, g * 512:(g + 1) * 512],
                            start=(dc == 0), stop=(dc == DC - 1))
                    nc.scalar.copy(kt_g[g][:, oc, :], ps[:, :])
                if g == 0:
                    nc.sync.dma_start(out=mask_sb[:, :, :, :],
                                      in_=mask_d[:, :, :, :])
                    for h in range(2):
                        nc.sync.dma_start(out=wv_h[h][:, :, :],
                                          in_=wv_d[:, h * HC:(h + 1) * HC, :])
                        nc.sync.dma_start(
                            out=xv_h[h][:, :, :],
                            in_=xv_d[:, h * (NT // 2):(h + 1) * (NT // 2), :])
                for kt in range(4 * g, 4 * g + 4):
                    emit_scoresT(kt)
                    if kt % 2 == 1:
                        emit_rest(kt // 2)

    nc.compile()
    return nc


def _pack(matT):
    """[D, W] (transposed operand) -> [P, DC, W] chunk layout, bf16."""
    d, w = matT.shape
    return np.ascontiguousarray(
        matT.reshape(d // P, P, w).transpose(1, 0, 2)).astype(BF16)


def shard_inputs(x, Wq, Wk, Wv):
    x = np.asarray(x, dtype=np.float32)
    wqT = _pack(np.asarray(Wq, np.float32).T)            # [P, DC, D]
    wqT = np.ascontiguousarray(                          # [P, 3, DC, 256]
        wqT.reshape(P, DC, 3, 256).transpose(0, 2, 1, 3))
    wkT = _pack(np.asarray(Wk, np.float32).T)
    wvT = _pack(np.asarray(Wv, np.float32).T)
    ident = np.eye(P, dtype=BF16)
    in_maps = []
    for c in range(N_CORES):
        b, side = divmod(c, 2)
        qtiles = SIDE_A if side == 0 else SIDE_B
        xb = x[b]                                    # [S, D]
        xkvT = _pack(np.ascontiguousarray(xb.T))
        xvR = np.empty((NT, P, XW), BF16)            # row-major + ones col
        xvR[:, :, :D] = xb.astype(BF16).reshape(NT, P, D)
        xvR[:, :, D] = BF16(1.0)
        xvR = np.ascontiguousarray(xvR.transpose(1, 0, 2))   # [P, NT, XW]
        xq = np.concatenate([xb[t * P:(t + 1) * P] for t in qtiles], axis=0)
        xqT = _pack(np.ascontiguousarray(xq.T))          # [P, DC, QROWS]
        xqT = np.ascontiguousarray(                      # [P, 2, DC, 512]
            xqT.reshape(P, DC, 2, 512).transpose(0, 2, 1, 3))
        # transposed mask [k-part, slot, which-tile, q-col]
        mask = np.empty((NSLOT, 2, P, P), np.float32)
        for s, t in enumerate(qtiles):
            qidx = t * P + np.arange(P)[None, :]         # query global
            for j in range(2):
                kt = CAP[s] - 2 + j
                kidx = kt * P + np.arange(P)[:, None]    # key global
                mask[s, j] = np.where(kidx <= qidx, 0.0, -1e30)
        # dram layout [P, NSLOT, 2, P]
        mask = np.ascontiguousarray(
            mask.transpose(2, 0, 1, 3)).astype(BF16)
        in_maps.append({"xqT": xqT, "xkvT": xkvT, "xvR": xvR, "wqT": wqT,
                        "wkT": wkT, "wvT": wvT, "maskT": mask,
                        "ident": ident})
    return in_maps


def unshard(results):
    out = np.empty((B, S, D), np.float32)
    for c in range(N_CORES):
        b, side = divmod(c, 2)
        qtiles = SIDE_A if side == 0 else SIDE_B
        oc = np.asarray(results[c]["out"]).astype(np.float32)
        for s, t in enumerate(qtiles):
            out[b, t * P:(t + 1) * P] = oc[s * P:(s + 1) * P]
    return out


def run(inputs, trace=False, trace_cores=None):
    """Run on hardware; returns (output, BassKernelResults)."""
    global _NC
    if _NC is None:
        _NC = build()
    in_maps = shard_inputs(inputs["x"], inputs["Wq"], inputs["Wk"],
                           inputs["Wv"])
    res = run_bass_kernel_spmd(_NC, in_maps, core_ids=list(range(N_CORES)),
                               trace=trace, trace_cores=trace_cores)
    return unshard(res.results), res


def kernel(x, Wq, Wk, Wv):
    out, _ = run({"x": x, "Wq": Wq, "Wk": Wk, "Wv": Wv})
    return out


# revision 29
# speedup vs baseline: 1.0105x; 1.0089x over previous
"""Trainium2 Bass kernel for single-head causal attention.

Problem: x:[4,2048,768], Wq/Wk/Wv:[768,768] (torch-Linear layout, y = x @ W.T),
out = causal_softmax(q k^T / sqrt(768)) @ v, all float32.

Sharding (8 NeuronCores, no collectives):
  - core pair (2b, 2b+1) handles batch b.
  - per batch, the 16 query tiles of 128 rows are split between the pair as
    {0,3,4,7,8,11,12,15} and {1,2,5,6,9,10,13,14}. Sorted by causal length
    those are {1,4,5,8,9,12,13,16} and {2,3,6,7,10,11,14,15} key-tiles, so
    both sides fit the same static per-slot key budget {2,4,...,16}: the one
    SPMD graph processes 8 query tiles whose key ranges are padded by at most
    one 128-tile (+6% flops) and the pad/diagonal is handled by a host-
    provided additive mask.
  - scores are computed TRANSPOSED (kT stationary, qT moving -> [k, q] in
    PSUM): exp output probsT[k, q] is directly the stationary operand the
    probs @ x matmul needs, so no probability transposes at all.  The softmax
    denominator comes from a ones-column appended to the row-major x operand
    (Y[:, 768] = sum of probs), and 1/l is folded into the Y -> SBUF copies.
  - out = (probs @ x) @ Wv^T (saves the full-seq V projection); only the
    [q, d] -> [d, q] transpose of Y runs on the TensorEngine (6 per slot).
  - host pre-transposes/packs inputs to bf16 and supplies the 128x128
    identity, so the device never builds constants on the critical path; PE
    warm-up runs on a memset-zeros tile starting immediately.
"""

import math
import os
import sys

import numpy as np

if not any(os.path.isdir(os.path.join(p, "concourse")) for p in sys.path):
    sys.path.insert(0, "/opt/trn_rl_repo")

import concourse.bass as bass  # noqa: E402
import concourse.mybir as mybir  # noqa: E402
from concourse import bacc, tile  # noqa: E402
from concourse.bass_utils import run_bass_kernel_spmd  # noqa: E402

import ml_dtypes  # noqa: E402

B, S, D = 4, 2048, 768
P = 128
NT = S // P          # 16 key tiles per batch
DC = D // P          # 6 contraction chunks
NSLOT = 8            # query tiles per core
QROWS = NSLOT * P    # 1024 query rows per core
N_CORES = 8
SCALE = 1.0 / math.sqrt(D)
XW = D + 1           # row-major x width incl. the ones column

SIDE_A = [0, 3, 4, 7, 8, 11, 12, 15]   # causal lengths 1,4,5,8,9,12,13,16
SIDE_B = [1, 2, 5, 6, 9, 10, 13, 14]   # causal lengths 2,3,6,7,10,11,14,15
CAP = [2, 4, 6, 8, 10, 12, 14, 16]     # static key tiles per slot (>= real)

BF16 = ml_dtypes.bfloat16

_NC = None


def ktw(kt):
    """Query-column span of key tile kt: slots s >= kt//2 need it."""
    return QROWS - (kt // 2) * P


def build():
    """Build + compile the single SPMD graph run by all 8 cores."""
    f32 = mybir.dt.float32
    bf16 = mybir.dt.bfloat16

    nc = bacc.Bacc("TRN2", target_bir_lowering=False, debug=False,
                   num_devices=N_CORES)

    # inputs come pre-packed as [P, chunk, width] (host layout transform)
    xq_d = nc.dram_tensor("xqT", [P, 2, DC, 512], bf16,
                          kind="ExternalInput").ap()
    xkv_d = nc.dram_tensor("xkvT", [P, DC, S], bf16,
                           kind="ExternalInput").ap()
    xv_d = nc.dram_tensor("xvR", [P, NT, XW], bf16,
                          kind="ExternalInput").ap()
    wq_d = nc.dram_tensor("wqT", [P, 3, DC, 256], bf16,
                          kind="ExternalInput").ap()
    wk_d = nc.dram_tensor("wkT", [P, DC, D], bf16, kind="ExternalInput").ap()
    wv_d = nc.dram_tensor("wvT", [P, DC, D], bf16, kind="ExternalInput").ap()
    mask_d = nc.dram_tensor("maskT", [P, NSLOT, 2, P], bf16,
                            kind="ExternalInput").ap()
    id_d = nc.dram_tensor("ident", [P, P], bf16, kind="ExternalInput").ap()
    out_d = nc.dram_tensor("out", [QROWS, D], bf16,
                           kind="ExternalOutput").ap()

    with tile.TileContext(nc) as tc:
        with (
            tc.tile_pool(name="const", bufs=1) as const,
            tc.tile_pool(name="osb", bufs=2) as osb_pool,
            tc.tile_pool(name="yt", bufs=2) as yt_pool,
            tc.tile_pool(name="small", bufs=2) as small,
            tc.tile_pool(name="ps_s", bufs=2, space="PSUM") as ps_s,
            tc.tile_pool(name="ps_tr", bufs=2, space="PSUM") as ps_tr,
            tc.tile_pool(name="ps_o", bufs=2, space="PSUM") as ps_o,
        ):
            # ---- persistent SBUF tensors, halves so DMA overlaps compute
            HC = DC // 2
            wq_p = [const.tile([P, DC, 256], bf16, tag=f"wqp{i}",
                               name=f"wqp{i}") for i in range(3)]
            wk_h = [const.tile([P, HC, D], bf16, tag=f"wkh{h}", name=f"wkh{h}")
                    for h in range(2)]
            wv_h = [const.tile([P, HC, D], bf16, tag=f"wvh{h}", name=f"wvh{h}")
                    for h in range(2)]
            xq_g = [const.tile([P, DC, 512], bf16, tag=f"xqg{g}",
                               name=f"xqg{g}") for g in range(2)]
            xkv_h = [const.tile([P, HC, S], bf16, tag=f"xkvh{h}",
                                name=f"xkvh{h}") for h in range(2)]

            def chunk(tiles, dc):
                return tiles[dc // HC][:, dc % HC, :]

            wk_c = [chunk(wk_h, c) for c in range(DC)]
            wv_c = [chunk(wv_h, c) for c in range(DC)]
            xkv_c = [chunk(xkv_h, c) for c in range(DC)]
            mask_sb = const.tile([P, NSLOT, 2, P], bf16, tag="mask")
            ident = const.tile([P, P], bf16, tag="ident")
            zeros = const.tile([P, 512], bf16, tag="zeros")
            qt_sb = const.tile([P, DC, QROWS], bf16, tag="qt")
            kt_g = [const.tile([P, DC, 512], bf16, tag=f"ktg{g}",
                               name=f"ktg{g}") for g in range(S // 512)]
            xv_h = [const.tile([P, NT // 2, XW], bf16, tag=f"xvh{h}",
                               name=f"xvh{h}") for h in range(2)]
            probsT = [const.tile([P, ktw(kt)], bf16, tag=f"pT{kt}",
                                 name=f"pT{kt}") for kt in range(NT)]

            # priority-ordered input DMAs: Q-projection group first
            nc.sync.dma_start(out=xq_g[0][:, :, :], in_=xq_d[:, 0, :, :])
            for i in range(3):
                nc.sync.dma_start(out=wq_p[i][:, :, :], in_=wq_d[:, i, :, :])
            nc.sync.dma_start(out=xq_g[1][:, :, :], in_=xq_d[:, 1, :, :])
            nc.sync.dma_start(out=ident[:, :], in_=id_d[:, :])

            # HAM warm-up on a zeroed tile: PE busy from ~0.3us so the real
            # matmuls run at 2.4GHz as soon as their inputs land.
            # 42 x 128-wide on zeros: ~3.4us cold flips HAM to 2.4GHz, the
            # rest bridges to the first input DMA landing (~12.5us)
            nc.gpsimd.memset(zeros[:, :], 0.0)
            warm = ps_s.tile([P, 512], f32, tag="mm512", name="warm")
            for _ in range(42):
                nc.tensor.matmul(warm[:, 0:P], zeros[:, 0:P], zeros[:, 0:P],
                                 start=True, stop=True)

            # ---- qT[o,q] projection (group-major: starts on first DMAs)
            for g in range(QROWS // 512):
                for oc in range(DC):
                    ps = ps_s.tile([P, 512], f32, tag="mm512")
                    for dc in range(DC):
                        nc.tensor.matmul(
                            ps[:, :],
                            wq_p[oc // 2][:, dc,
                                          (oc % 2) * P:(oc % 2 + 1) * P],
                            xq_g[g][:, dc, :],
                            start=(dc == 0), stop=(dc == DC - 1))
                    nc.scalar.copy(qt_sb[:, oc, g * 512:(g + 1) * 512],
                                   ps[:, :])

            for h in range(2):
                nc.sync.dma_start(out=xkv_h[h][:, :, :],
                                  in_=xkv_d[:, h * HC:(h + 1) * HC, :])
                nc.sync.dma_start(out=wk_h[h][:, :, :],
                                  in_=wk_d[:, h * HC:(h + 1) * HC, :])

            def gsplit(w):
                """Split w (multiple of 128) into <=512 parts, balanced in
                128-multiples: narrow matmuls are LDWEIGHTS-bound, so 384+384
                beats 512+256."""
                parts = (w + 511) // 512
                tiles = w // P
                out, acc = [], 0
                for i in range(parts):
                    t = (tiles * (i + 1)) // parts - (tiles * i) // parts
                    out.append((acc, t * P))
                    acc += t * P
                return out

            def emit_scoresT(kt):
                """scoresT[k, q] for key tile kt over q-cols [qlo, QROWS)."""
                qlo = (kt // 2) * P
                w = QROWS - qlo
                sm, j = kt // 2, kt % 2   # the one masked slot for this kt
                for off0, cw in gsplit(w):
                    c0 = qlo + off0
                    ps = ps_s.tile([P, 512], f32, tag="mm512",
                                   name=f"st{kt}_{off0}")
                    for oc in range(DC):
                        nc.tensor.matmul(
                            ps[:, :cw],
                            kt_g[kt // 4][:, oc, (kt % 4) * P:(kt % 4 + 1) * P],
                            qt_sb[:, oc, c0:c0 + cw],
                            start=(oc == 0), stop=(oc == DC - 1))
                    if c0 <= sm * P < c0 + cw:
                        off = sm * P - c0
                        nc.vector.tensor_add(ps[:, off:off + P],
                                             ps[:, off:off + P],
                                             mask_sb[:, sm, j, :])
                    nc.scalar.activation(
                        probsT[kt][:, c0 - qlo:c0 - qlo + cw], ps[:, :cw],
                        mybir.ActivationFunctionType.Exp, scale=SCALE)

            def emit_rest(s):
                """AV + (Y @ Wv^T)/l + output DMA for slot s."""
                L = CAP[s]
                # PSUM bank0 = Y[384:768] + l (385 cols), bank1 = Y[0:384]:
                # balanced 385/384 groups, neither crossing a bank boundary
                out_ps = ps_o.tile([P, 896], f32, tag="mmout",
                                   name=f"ops{s}")
                for kt in range(L):
                    pT = probsT[kt][:, s * P - (kt // 2) * P:
                                    (s + 1) * P - (kt // 2) * P]
                    xv = xv_h[kt // (NT // 2)][:, kt % (NT // 2), :]
                    nc.tensor.matmul(out_ps[:, 0:385],
                                     pT, xv[:, 384:XW],
                                     start=(kt == 0), stop=(kt == L - 1))
                    nc.tensor.matmul(out_ps[:, 512:896],
                                     pT, xv[:, 0:384],
                                     start=(kt == 0), stop=(kt == L - 1))
                rinv = small.tile([P, 1], f32, tag="rinv", name=f"rinv{s}")
                nc.vector.reciprocal(rinv[:, :], out_ps[:, 384:385])
                # unnormalized Y in bf16 (plain copies keep the chain short);
                # 1/l is applied on the final output copies instead
                y_hi = osb_pool.tile([P, 384], bf16, tag="yhi", name=f"yhi{s}")
                y_lo = osb_pool.tile([P, 384], bf16, tag="ylo", name=f"ylo{s}")
                nc.vector.tensor_copy(y_hi[:, :], out_ps[:, 0:384])
                nc.scalar.copy(y_lo[:, :], out_ps[:, 512:896])
                ytT = yt_pool.tile([P, D], bf16, tag="ytT", name=f"ytT{s}")
                for kg in range(2):      # hi half first: its operand lands
                    tp = ps_tr.tile([P, 384], bf16, tag="tr",
                                    name=f"ytp{s}")  # first
                    ysrc_t = y_hi if kg == 0 else y_lo
                    base = 3 if kg == 0 else 0
                    for j in range(3):
                        nc.tensor.transpose(tp[:, j * P:(j + 1) * P],
                                            ysrc_t[:, j * P:(j + 1) * P],
                                            ident[:, :])
                    nc.vector.tensor_copy(ytT[:, base * P:base * P + 384],
                                          tp[:, 0:384])
                out2_ps = ps_o.tile([P, 896], f32, tag="mmout",
                                    name=f"o2ps{s}")
                out_sb = osb_pool.tile([P, D], bf16, tag="osb", name=f"osb{s}")
                # accumulate hi chunks (3,4,5) first: they transpose first,
                # so the final matmuls start before the lo half is copied
                dcs = [3, 4, 5, 0, 1, 2]
                for i, dc in enumerate(dcs):
                    nc.tensor.matmul(out2_ps[:, 0:384],
                                     ytT[:, dc * P:(dc + 1) * P],
                                     wv_c[dc][:, 384:D],
                                     start=(i == 0), stop=(i == DC - 1))
                nc.scalar.mul(out_sb[:, 384:D], out2_ps[:, 0:384],
                              rinv[:, :])
                nc.sync.dma_start(out=out_d[s * P:(s + 1) * P, 384:D],
                                  in_=out_sb[:, 384:D])
                for i, dc in enumerate(dcs):
                    nc.tensor.matmul(out2_ps[:, 512:896],
                                     ytT[:, dc * P:(dc + 1) * P],
                                     wv_c[dc][:, 0:384],
                                     start=(i == 0), stop=(i == DC - 1))
                nc.vector.tensor_scalar_mul(out_sb[:, 0:384],
                                            out2_ps[:, 512:896], rinv[:, :])
                nc.sync.dma_start(out=out_d[s * P:(s + 1) * P, 0:384],
                                  in_=out_sb[:, 0:384])

            # kT[o,k] projection group-major, scoresT + attention interleaved
            for g in range(S // 512):
                for oc in range(DC):
                    ps = ps_s.tile([P, 512], f32, tag="mm512",
                                   name=f"ktps{g}_{oc}")
                    for dc in range(DC):
                        nc.tensor.matmul(
                            ps[:, :],
                            wk_c[dc][:, oc * P:(oc + 1) * P],
                            xkv_c[dc][:, g * 512:(g + 1) * 512],
                            start=(dc == 0), stop=(dc == DC - 1))
                    nc.scalar.copy(kt_g[g][:, oc, :], ps[:, :])
                if g == 0:
                    nc.sync.dma_start(out=mask_sb[:, :, :, :],
                                      in_=mask_d[:, :, :, :])
                    for h in range(2):
                        nc.sync.dma_start(out=wv_h[h][:, :, :],
                                          in_=wv_d[:, h * HC:(h + 1) * HC, :])
                        nc.sync.dma_start(
                            out=xv_h[h][:, :, :],
                            in_=xv_d[:, h * (NT // 2):(h + 1) * (NT // 2), :])
                for kt in range(4 * g, 4 * g + 4):
                    emit_scoresT(kt)
                    if kt % 2 == 1:
                        emit_rest(kt // 2)

    nc.compile()
    return nc


def _pack(matT):
    """[D, W] (transposed operand) -> [P, DC, W] chunk layout, bf16."""
    d, w = matT.shape
    return np.ascontiguousarray(
        matT.reshape(d // P, P, w).transpose(1, 0, 2)).astype(BF16)


def shard_inputs(x, Wq, Wk, Wv):
    x = np.asarray(x, dtype=np.float32)
    wqT = _pack(np.asarray(Wq, np.float32).T)            # [P, DC, D]
    wqT = np.ascontiguousarray(                          # [P, 3, DC, 256]
        wqT.reshape(P, DC, 3, 256).transpose(0, 2, 1, 3))
    wkT = _pack(np.asarray(Wk, np.float32).T)
    wvT = _pack(np.asarray(Wv, np.float32).T)
    ident = np.eye(P, dtype=BF16)
    in_maps = []
    for c in range(N_CORES):
        b, side = divmod(c, 2)
        qtiles = SIDE_A if side == 0 else SIDE_B
        xb = x[b]                                    # [S, D]
        xkvT = _pack(np.ascontiguousarray(xb.T))
        xvR = np.empty((NT, P, XW), BF16)            # row-major + ones col
        xvR[:, :, :D] = xb.astype(BF16).reshape(NT, P, D)
        xvR[:, :, D] = BF16(1.0)
        xvR = np.ascontiguousarray(xvR.transpose(1, 0, 2))   # [P, NT, XW]
        xq = np.concatenate([xb[t * P:(t + 1) * P] for t in qtiles], axis=0)
        xqT = _pack(np.ascontiguousarray(xq.T))          # [P, DC, QROWS]
        xqT = np.ascontiguousarray(                      # [P, 2, DC, 512]
            xqT.reshape(P, DC, 2, 512).transpose(0, 2, 1, 3))
        # transposed mask [k-part, slot, which-tile, q-col]
        mask = np.empty((NSLOT, 2, P, P), np.float32)
        for s, t in enumerate(qtiles):
            qidx = t * P + np.arange(P)[None, :]         # query global
            for j in range(2):
                kt = CAP[s] - 2 + j
                kidx = kt * P + np.arange(P)[:, None]    # key global
                mask[s, j] = np.where(kidx <= qidx, 0.0, -1e30)
        # dram layout [P, NSLOT, 2, P]
        mask = np.ascontiguousarray(
            mask.transpose(2, 0, 1, 3)).astype(BF16)
        in_maps.append({"xqT": xqT, "xkvT": xkvT, "xvR": xvR, "wqT": wqT,
                        "wkT": wkT, "wvT": wvT, "maskT": mask,
                        "ident": ident})
    return in_maps


def unshard(results):
    out = np.empty((B, S, D), np.float32)
    for c in range(N_CORES):
        b, side = divmod(c, 2)
        qtiles = SIDE_A if side == 0 else SIDE_B
        oc = np.asarray(results[c]["out"]).astype(np.float32)
        for s, t in enumerate(qtiles):
            out[b, t * P:(t + 1) * P] = oc[s * P:(s + 1) * P]
    return out


def run(inputs, trace=False, trace_cores=None):
    """Run on hardware; returns (output, BassKernelResults)."""
    global _NC
    if _NC is None:
        _NC = build()
    in_maps = shard_inputs(inputs["x"], inputs["Wq"], inputs["Wk"],
                           inputs["Wv"])
    res = run_bass_kernel_spmd(_NC, in_maps, core_ids=list(range(N_CORES)),
                               trace=trace, trace_cores=trace_cores)
    return unshard(res.results), res


def kernel(x, Wq, Wk, Wv):
    out, _ = run({"x": x, "Wq": Wq, "Wk": Wk, "Wv": Wv})
    return out


# revision 30
# speedup vs baseline: 1.0142x; 1.0037x over previous
"""Trainium2 Bass kernel for single-head causal attention.

Problem: x:[4,2048,768], Wq/Wk/Wv:[768,768] (torch-Linear layout, y = x @ W.T),
out = causal_softmax(q k^T / sqrt(768)) @ v, all float32.

Sharding (8 NeuronCores, no collectives):
  - core pair (2b, 2b+1) handles batch b.
  - per batch, the 16 query tiles of 128 rows are split between the pair as
    {0,3,4,7,8,11,12,15} and {1,2,5,6,9,10,13,14}. Sorted by causal length
    those are {1,4,5,8,9,12,13,16} and {2,3,6,7,10,11,14,15} key-tiles, so
    both sides fit the same static per-slot key budget {2,4,...,16}: the one
    SPMD graph processes 8 query tiles whose key ranges are padded by at most
    one 128-tile (+6% flops) and the pad/diagonal is handled by a host-
    provided additive mask.
  - scores are computed TRANSPOSED (kT stationary, qT moving -> [k, q] in
    PSUM): exp output probsT[k, q] is directly the stationary operand the
    probs @ x matmul needs, so no probability transposes at all.  The softmax
    denominator comes from a ones-column appended to the row-major x operand
    (Y[:, 768] = sum of probs), and 1/l is folded into the Y -> SBUF copies.
  - out = (probs @ x) @ Wv^T (saves the full-seq V projection); only the
    [q, d] -> [d, q] transpose of Y runs on the TensorEngine (6 per slot).
  - host pre-transposes/packs inputs to bf16 and supplies the 128x128
    identity, so the device never builds constants on the critical path; PE
    warm-up runs on a memset-zeros tile starting immediately.
"""

import math
import os
import sys

import numpy as np

if not any(os.path.isdir(os.path.join(p, "concourse")) for p in sys.path):
    sys.path.insert(0, "/opt/trn_rl_repo")

import concourse.bass as bass  # noqa: E402
import concourse.mybir as mybir  # noqa: E402
from concourse import bacc, tile  # noqa: E402
from concourse.bass_utils import run_bass_kernel_spmd  # noqa: E402

import ml_dtypes  # noqa: E402

B, S, D = 4, 2048, 768
P = 128
NT = S // P          # 16 key tiles per batch
DC = D // P          # 6 contraction chunks
NSLOT = 8            # query tiles per core
QROWS = NSLOT * P    # 1024 query rows per core
N_CORES = 8
SCALE = 1.0 / math.sqrt(D)
XW = D + 1           # row-major x width incl. the ones column

SIDE_A = [0, 3, 4, 7, 8, 11, 12, 15]   # causal lengths 1,4,5,8,9,12,13,16
SIDE_B = [1, 2, 5, 6, 9, 10, 13, 14]   # causal lengths 2,3,6,7,10,11,14,15
CAP = [2, 4, 6, 8, 10, 12, 14, 16]     # static key tiles per slot (>= real)

BF16 = ml_dtypes.bfloat16

_NC = None


def ktw(kt):
    """Query-column span of key tile kt: slots s >= kt//2 need it."""
    return QROWS - (kt // 2) * P


def build():
    """Build + compile the single SPMD graph run by all 8 cores."""
    f32 = mybir.dt.float32
    bf16 = mybir.dt.bfloat16

    nc = bacc.Bacc("TRN2", target_bir_lowering=False, debug=False,
                   num_devices=N_CORES)

    # inputs come pre-packed as [P, chunk, width] (host layout transform)
    xq_d = nc.dram_tensor("xqT", [P, 2, DC, 512], bf16,
                          kind="ExternalInput").ap()
    xkv_d = nc.dram_tensor("xkvT", [P, DC, S], bf16,
                           kind="ExternalInput").ap()
    xv_d = nc.dram_tensor("xvR", [P, NT, XW], bf16,
                          kind="ExternalInput").ap()
    wq_d = nc.dram_tensor("wqT", [P, 3, DC, 256], bf16,
                          kind="ExternalInput").ap()
    wk_d = nc.dram_tensor("wkT", [P, DC, D], bf16, kind="ExternalInput").ap()
    wv_d = nc.dram_tensor("wvT", [P, DC, D], bf16, kind="ExternalInput").ap()
    mask_d = nc.dram_tensor("maskT", [P, NSLOT, 2, P], bf16,
                            kind="ExternalInput").ap()
    id_d = nc.dram_tensor("ident", [P, P], bf16, kind="ExternalInput").ap()
    out_d = nc.dram_tensor("out", [QROWS, D], bf16,
                           kind="ExternalOutput").ap()

    with tile.TileContext(nc) as tc:
        with (
            tc.tile_pool(name="const", bufs=1) as const,
            tc.tile_pool(name="osb", bufs=2) as osb_pool,
            tc.tile_pool(name="yt", bufs=2) as yt_pool,
            tc.tile_pool(name="small", bufs=2) as small,
            tc.tile_pool(name="ps_s", bufs=2, space="PSUM") as ps_s,
            tc.tile_pool(name="ps_tr", bufs=2, space="PSUM") as ps_tr,
            tc.tile_pool(name="ps_o", bufs=2, space="PSUM") as ps_o,
        ):
            # ---- persistent SBUF tensors, halves so DMA overlaps compute
            HC = DC // 2
            wq_p = [const.tile([P, DC, 256], bf16, tag=f"wqp{i}",
                               name=f"wqp{i}") for i in range(3)]
            wk_h = [const.tile([P, HC, D], bf16, tag=f"wkh{h}", name=f"wkh{h}")
                    for h in range(2)]
            wv_h = [const.tile([P, HC, D], bf16, tag=f"wvh{h}", name=f"wvh{h}")
                    for h in range(2)]
            xq_g = [const.tile([P, DC, 512], bf16, tag=f"xqg{g}",
                               name=f"xqg{g}") for g in range(2)]
            xkv_h = [const.tile([P, HC, S], bf16, tag=f"xkvh{h}",
                                name=f"xkvh{h}") for h in range(2)]

            def chunk(tiles, dc):
                return tiles[dc // HC][:, dc % HC, :]

            wk_c = [chunk(wk_h, c) for c in range(DC)]
            wv_c = [chunk(wv_h, c) for c in range(DC)]
            xkv_c = [chunk(xkv_h, c) for c in range(DC)]
            mask_sb = const.tile([P, NSLOT, 2, P], bf16, tag="mask")
            ident = const.tile([P, P], bf16, tag="ident")
            zeros = const.tile([P, 512], bf16, tag="zeros")
            qt_sb = const.tile([P, DC, QROWS], bf16, tag="qt")
            kt_g = [const.tile([P, DC, 512], bf16, tag=f"ktg{g}",
                               name=f"ktg{g}") for g in range(S // 512)]
            xv_h = [const.tile([P, NT // 2, XW], bf16, tag=f"xvh{h}",
                               name=f"xvh{h}") for h in range(2)]
            probsT = [const.tile([P, ktw(kt)], bf16, tag=f"pT{kt}",
                                 name=f"pT{kt}") for kt in range(NT)]

            # priority-ordered input DMAs: Q-projection group first
            nc.sync.dma_start(out=xq_g[0][:, :, :], in_=xq_d[:, 0, :, :])
            for i in range(3):
                nc.sync.dma_start(out=wq_p[i][:, :, :], in_=wq_d[:, i, :, :])
            nc.sync.dma_start(out=xq_g[1][:, :, :], in_=xq_d[:, 1, :, :])
            nc.sync.dma_start(out=ident[:, :], in_=id_d[:, :])

            # HAM warm-up on a zeroed tile: PE busy from ~0.3us so the real
            # matmuls run at 2.4GHz as soon as their inputs land.
            # 42 x 128-wide on zeros: ~3.4us cold flips HAM to 2.4GHz, the
            # rest bridges to the first input DMA landing (~12.5us)
            nc.gpsimd.memset(zeros[:, :], 0.0)
            warm = ps_s.tile([P, 512], f32, tag="mm512", name="warm")
            for _ in range(42):
                nc.tensor.matmul(warm[:, 0:P], zeros[:, 0:P], zeros[:, 0:P],
                                 start=True, stop=True)

            # ---- qT[o,q] projection (group-major: starts on first DMAs)
            for g in range(QROWS // 512):
                for oc in range(DC):
                    ps = ps_s.tile([P, 512], f32, tag="mm512")
                    for dc in range(DC):
                        nc.tensor.matmul(
                            ps[:, :],
                            wq_p[oc // 2][:, dc,
                                          (oc % 2) * P:(oc % 2 + 1) * P],
                            xq_g[g][:, dc, :],
                            start=(dc == 0), stop=(dc == DC - 1))
                    nc.scalar.copy(qt_sb[:, oc, g * 512:(g + 1) * 512],
                                   ps[:, :])

            for h in range(2):
                nc.sync.dma_start(out=xkv_h[h][:, :, :],
                                  in_=xkv_d[:, h * HC:(h + 1) * HC, :])
                nc.sync.dma_start(out=wk_h[h][:, :, :],
                                  in_=wk_d[:, h * HC:(h + 1) * HC, :])

            def gsplit(w):
                """Split w (multiple of 128) into <=512 parts, balanced in
                128-multiples: narrow matmuls are LDWEIGHTS-bound, so 384+384
                beats 512+256."""
                parts = (w + 511) // 512
                tiles = w // P
                out, acc = [], 0
                for i in range(parts):
                    t = (tiles * (i + 1)) // parts - (tiles * i) // parts
                    out.append((acc, t * P))
                    acc += t * P
                return out

            def emit_scoresT(kt):
                """scoresT[k, q] for key tile kt over q-cols [qlo, QROWS)."""
                qlo = (kt // 2) * P
                w = QROWS - qlo
                sm, j = kt // 2, kt % 2   # the one masked slot for this kt
                for off0, cw in gsplit(w):
                    c0 = qlo + off0
                    ps = ps_s.tile([P, 512], f32, tag="mm512",
                                   name=f"st{kt}_{off0}")
                    for oc in range(DC):
                        nc.tensor.matmul(
                            ps[:, :cw],
                            kt_g[kt // 4][:, oc, (kt % 4) * P:(kt % 4 + 1) * P],
                            qt_sb[:, oc, c0:c0 + cw],
                            start=(oc == 0), stop=(oc == DC - 1))
                    if c0 <= sm * P < c0 + cw:
                        off = sm * P - c0
                        nc.vector.tensor_add(ps[:, off:off + P],
                                             ps[:, off:off + P],
                                             mask_sb[:, sm, j, :])
                    nc.scalar.activation(
                        probsT[kt][:, c0 - qlo:c0 - qlo + cw], ps[:, :cw],
                        mybir.ActivationFunctionType.Exp, scale=SCALE)

            def emit_rest(s):
                """AV + (Y @ Wv^T)/l + output DMA for slot s."""
                L = CAP[s]
                # PSUM bank0 = Y[384:768] + l (385 cols), bank1 = Y[0:384]:
                # balanced 385/384 groups, neither crossing a bank boundary
                out_ps = ps_o.tile([P, 896], f32, tag="mmout",
                                   name=f"ops{s}")
                # complete group A (Y[384:768] + l) before starting group B:
                # the rinv / y_hi / hi-transpose chain then overlaps all of
                # B's matmuls instead of serializing after them
                for kt in range(L):
                    pT = probsT[kt][:, s * P - (kt // 2) * P:
                                    (s + 1) * P - (kt // 2) * P]
                    xv = xv_h[kt // (NT // 2)][:, kt % (NT // 2), :]
                    nc.tensor.matmul(out_ps[:, 0:385],
                                     pT, xv[:, 384:XW],
                                     start=(kt == 0), stop=(kt == L - 1))
                for kt in range(L):
                    pT = probsT[kt][:, s * P - (kt // 2) * P:
                                    (s + 1) * P - (kt // 2) * P]
                    xv = xv_h[kt // (NT // 2)][:, kt % (NT // 2), :]
                    nc.tensor.matmul(out_ps[:, 512:896],
                                     pT, xv[:, 0:384],
                                     start=(kt == 0), stop=(kt == L - 1))
                rinv = small.tile([P, 1], f32, tag="rinv", name=f"rinv{s}")
                nc.vector.reciprocal(rinv[:, :], out_ps[:, 384:385])
                # unnormalized Y in bf16 (plain copies keep the chain short);
                # 1/l is applied on the final output copies instead
                y_hi = osb_pool.tile([P, 384], bf16, tag="yhi", name=f"yhi{s}")
                y_lo = osb_pool.tile([P, 384], bf16, tag="ylo", name=f"ylo{s}")
                nc.vector.tensor_copy(y_hi[:, :], out_ps[:, 0:384])
                nc.scalar.copy(y_lo[:, :], out_ps[:, 512:896])
                ytT = yt_pool.tile([P, D], bf16, tag="ytT", name=f"ytT{s}")
                for kg in range(2):      # hi half first: its operand lands
                    tp = ps_tr.tile([P, 384], bf16, tag="tr",
                                    name=f"ytp{s}")  # first
                    ysrc_t = y_hi if kg == 0 else y_lo
                    base = 3 if kg == 0 else 0
                    for j in range(3):
                        nc.tensor.transpose(tp[:, j * P:(j + 1) * P],
                                            ysrc_t[:, j * P:(j + 1) * P],
                                            ident[:, :])
                    nc.vector.tensor_copy(ytT[:, base * P:base * P + 384],
                                          tp[:, 0:384])
                out2_ps = ps_o.tile([P, 896], f32, tag="mmout",
                                    name=f"o2ps{s}")
                out_sb = osb_pool.tile([P, D], bf16, tag="osb", name=f"osb{s}")
                # accumulate hi chunks (3,4,5) first: they transpose first,
                # so the final matmuls start before the lo half is copied
                dcs = [3, 4, 5, 0, 1, 2]
                for i, dc in enumerate(dcs):
                    nc.tensor.matmul(out2_ps[:, 0:384],
                                     ytT[:, dc * P:(dc + 1) * P],
                                     wv_c[dc][:, 384:D],
                                     start=(i == 0), stop=(i == DC - 1))
                nc.scalar.mul(out_sb[:, 384:D], out2_ps[:, 0:384],
                              rinv[:, :])
                nc.sync.dma_start(out=out_d[s * P:(s + 1) * P, 384:D],
                                  in_=out_sb[:, 384:D])
                for i, dc in enumerate(dcs):
                    nc.tensor.matmul(out2_ps[:, 512:896],
                                     ytT[:, dc * P:(dc + 1) * P],
                                     wv_c[dc][:, 0:384],
                                     start=(i == 0), stop=(i == DC - 1))
                nc.vector.tensor_scalar_mul(out_sb[:, 0:384],
                                            out2_ps[:, 512:896], rinv[:, :])
                nc.sync.dma_start(out=out_d[s * P:(s + 1) * P, 0:384],
                                  in_=out_sb[:, 0:384])

            # kT[o,k] projection group-major, scoresT + attention interleaved
            for g in range(S // 512):
                for oc in range(DC):
                    ps = ps_s.tile([P, 512], f32, tag="mm512",
                                   name=f"ktps{g}_{oc}")
                    for dc in range(DC):
                        nc.tensor.matmul(
                            ps[:, :],
                            wk_c[dc][:, oc * P:(oc + 1) * P],
                            xkv_c[dc][:, g * 512:(g + 1) * 512],
                            start=(dc == 0), stop=(dc == DC - 1))
                    nc.scalar.copy(kt_g[g][:, oc, :], ps[:, :])
                if g == 0:
                    nc.sync.dma_start(out=mask_sb[:, :, :, :],
                                      in_=mask_d[:, :, :, :])
                    for h in range(2):
                        nc.sync.dma_start(out=wv_h[h][:, :, :],
                                          in_=wv_d[:, h * HC:(h + 1) * HC, :])
                        nc.sync.dma_start(
                            out=xv_h[h][:, :, :],
                            in_=xv_d[:, h * (NT // 2):(h + 1) * (NT // 2), :])
                for kt in range(4 * g, 4 * g + 4):
                    emit_scoresT(kt)
                    if kt % 2 == 1:
                        emit_rest(kt // 2)

    nc.compile()
    return nc


def _pack(matT):
    """[D, W] (transposed operand) -> [P, DC, W] chunk layout, bf16."""
    d, w = matT.shape
    return np.ascontiguousarray(
        matT.reshape(d // P, P, w).transpose(1, 0, 2)).astype(BF16)


def shard_inputs(x, Wq, Wk, Wv):
    x = np.asarray(x, dtype=np.float32)
    wqT = _pack(np.asarray(Wq, np.float32).T)            # [P, DC, D]
    wqT = np.ascontiguousarray(                          # [P, 3, DC, 256]
        wqT.reshape(P, DC, 3, 256).transpose(0, 2, 1, 3))
    wkT = _pack(np.asarray(Wk, np.float32).T)
    wvT = _pack(np.asarray(Wv, np.float32).T)
    ident = np.eye(P, dtype=BF16)
    in_maps = []
    for c in range(N_CORES):
        b, side = divmod(c, 2)
        qtiles = SIDE_A if side == 0 else SIDE_B
        xb = x[b]                                    # [S, D]
        xkvT = _pack(np.ascontiguousarray(xb.T))
        xvR = np.empty((NT, P, XW), BF16)            # row-major + ones col
        xvR[:, :, :D] = xb.astype(BF16).reshape(NT, P, D)
        xvR[:, :, D] = BF16(1.0)
        xvR = np.ascontiguousarray(xvR.transpose(1, 0, 2))   # [P, NT, XW]
        xq = np.concatenate([xb[t * P:(t + 1) * P] for t in qtiles], axis=0)
        xqT = _pack(np.ascontiguousarray(xq.T))          # [P, DC, QROWS]
        xqT = np.ascontiguousarray(                      # [P, 2, DC, 512]
            xqT.reshape(P, DC, 2, 512).transpose(0, 2, 1, 3))
        # transposed mask [k-part, slot, which-tile, q-col]
        mask = np.empty((NSLOT, 2, P, P), np.float32)
        for s, t in enumerate(qtiles):
            qidx = t * P + np.arange(P)[None, :]         # query global
            for j in range(2):
                kt = CAP[s] - 2 + j
                kidx = kt * P + np.arange(P)[:, None]    # key global
                mask[s, j] = np.where(kidx <= qidx, 0.0, -1e30)
        # dram layout [P, NSLOT, 2, P]
        mask = np.ascontiguousarray(
            mask.transpose(2, 0, 1, 3)).astype(BF16)
        in_maps.append({"xqT": xqT, "xkvT": xkvT, "xvR": xvR, "wqT": wqT,
                        "wkT": wkT, "wvT": wvT, "maskT": mask,
                        "ident": ident})
    return in_maps


def unshard(results):
    out = np.empty((B, S, D), np.float32)
    for c in range(N_CORES):
        b, side = divmod(c, 2)
        qtiles = SIDE_A if side == 0 else SIDE_B
        oc = np.asarray(results[c]["out"]).astype(np.float32)
        for s, t in enumerate(qtiles):
            out[b, t * P:(t + 1) * P] = oc[s * P:(s + 1) * P]
    return out


def run(inputs, trace=False, trace_cores=None):
    """Run on hardware; returns (output, BassKernelResults)."""
    global _NC
    if _NC is None:
        _NC = build()
    in_maps = shard_inputs(inputs["x"], inputs["Wq"], inputs["Wk"],
                           inputs["Wv"])
    res = run_bass_kernel_spmd(_NC, in_maps, core_ids=list(range(N_CORES)),
                               trace=trace, trace_cores=trace_cores)
    return unshard(res.results), res


def kernel(x, Wq, Wk, Wv):
    out, _ = run({"x": x, "Wq": Wq, "Wk": Wk, "Wv": Wv})
    return out
